# revision 6
# baseline (speedup 1.0000x reference)
import sys

if "/opt/trn_rl_repo" not in sys.path:
    sys.path.insert(0, "/opt/trn_rl_repo")

import numpy as np
import ml_dtypes

BF16 = ml_dtypes.bfloat16
B, S, H = 2, 2048, 4096
NH, NKV, D = 32, 8, 128
T = B * S
KBLK = H // 128  # 32
SCALE = float(D) ** -0.5
NCORES = 8

_NC = None


def build_nc():
    from concourse import bacc, tile, mybir

    dt = mybir.dt
    Act = mybir.ActivationFunctionType
    Alu = mybir.AluOpType
    import concourse.bass as bassmod

    nc = bacc.Bacc("TRN2", target_bir_lowering=False, debug=False, num_devices=NCORES)

    hsT_d = nc.dram_tensor("hsT", [128, KBLK, T], dt.bfloat16, kind="ExternalInput")
    cos_d = nc.dram_tensor("cosT", [128, T], dt.bfloat16, kind="ExternalInput")
    sin_d = nc.dram_tensor("sinT", [128, T], dt.bfloat16, kind="ExternalInput")
    wq_d = nc.dram_tensor("wq", [128, KBLK, 512], dt.bfloat16, kind="ExternalInput")
    wk_d = nc.dram_tensor("wk", [128, KBLK, 128], dt.bfloat16, kind="ExternalInput")
    wv_d = nc.dram_tensor("wv", [128, KBLK, 128], dt.bfloat16, kind="ExternalInput")
    wo_d = nc.dram_tensor("wo", [128, 4, H], dt.bfloat16, kind="ExternalInput")
    y_d = nc.dram_tensor("y", [T, H], dt.bfloat16, kind="ExternalOutput")
    recip_d = nc.dram_tensor("recip_scratch", [8, S], dt.float32)

    with tile.TileContext(nc) as tc:
        with tc.tile_pool(name="persist", bufs=1) as pp:
            Q_sb = pp.tile([128, 4, T], dt.bfloat16)
            K_sb = pp.tile([128, T], dt.bfloat16)
            V_sb = pp.tile([128, KBLK, 128], dt.bfloat16)
            OT_sb = pp.tile([128, 4, T], dt.bfloat16)
            ones = pp.tile([128, 1], dt.float32)
            nc.vector.memset(ones[:], 1.0)

            # ---------------- Phase A: QKV projections + RoPE ----------------
            with (
                tc.tile_pool(name="aw", bufs=1) as aw,
                tc.tile_pool(name="slabp", bufs=2) as slabp,
                tc.tile_pool(name="ascr", bufs=3) as ascr,
                tc.tile_pool(name="psa", bufs=1, space="PSUM") as psa,
            ):
                wq_sb = aw.tile([128, KBLK, 512], dt.bfloat16)
                wk_sb = aw.tile([128, KBLK, 128], dt.bfloat16)
                wv_sb = aw.tile([128, KBLK, 128], dt.bfloat16)
                cos_sb = aw.tile([128, T], dt.bfloat16)
                sin_sb = aw.tile([128, T], dt.bfloat16)
                nc.sync.dma_start(wq_sb[:], wq_d[:])
                nc.sync.dma_start(wk_sb[:], wk_d[:])
                nc.sync.dma_start(wv_sb[:], wv_d[:])
                nc.sync.dma_start(cos_sb[:], cos_d[:])
                nc.sync.dma_start(sin_sb[:], sin_d[:])

                def rope(dst3, hd, tb, src_ps):
                    # dst[:64] = x[:64]*cos[:64] - x[64:]*sin[:64]
                    # dst[64:] = x[64:]*cos[64:] + x[:64]*sin[64:]
                    c0, c1 = tb * 512, (tb + 1) * 512
                    tmpc = ascr.tile([128, 512], dt.float32)
                    tmps = ascr.tile([128, 512], dt.float32)
                    nc.vector.tensor_mul(tmpc[:], src_ps[:], cos_sb[:, c0:c1])
                    nc.vector.tensor_mul(
                        tmps[0:64, :], src_ps[64:128, :], sin_sb[0:64, c0:c1]
                    )
                    nc.vector.tensor_mul(
                        tmps[64:128, :], src_ps[0:64, :], sin_sb[64:128, c0:c1]
                    )
                    if hd is None:
                        d_lo = K_sb[0:64, c0:c1]
                        d_hi = K_sb[64:128, c0:c1]
                    else:
                        d_lo = Q_sb[0:64, hd, c0:c1]
                        d_hi = Q_sb[64:128, hd, c0:c1]
                    nc.vector.tensor_sub(d_lo, tmpc[0:64, :], tmps[0:64, :])
                    nc.vector.tensor_add(d_hi, tmpc[64:128, :], tmps[64:128, :])

                for tb in range(8):
                    c0 = tb * 512
                    qps = [
                        psa.tile([128, 512], dt.float32, name=f"qp{_h}")
                        for _h in range(4)
                    ]
                    kp = psa.tile([128, 512], dt.float32)
                    vtp = psa.tile([128, 512], dt.float32)
                    for half in range(2):
                        sl = slabp.tile([128, 16, 512], dt.bfloat16)
                        nc.sync.dma_start(
                            sl[:], hsT_d[:, half * 16 : (half + 1) * 16, c0 : c0 + 512]
                        )
                        for hd in range(4):
                            for kk in range(16):
                                k = half * 16 + kk
                                nc.tensor.matmul(
                                    qps[hd][:],
                                    wq_sb[:, k, hd * 128 : (hd + 1) * 128],
                                    sl[:, kk, :],
                                    start=(k == 0),
                                    stop=(k == KBLK - 1),
                                )
                        for kk in range(16):
                            k = half * 16 + kk
                            nc.tensor.matmul(
                                kp[:],
                                wk_sb[:, k, :],
                                sl[:, kk, :],
                                start=(k == 0),
                                stop=(k == KBLK - 1),
                            )
                        for kk in range(16):
                            k = half * 16 + kk
                            nc.tensor.matmul(
                                vtp[:],
                                wv_sb[:, k, :],
                                sl[:, kk, :],
                                start=(k == 0),
                                stop=(k == KBLK - 1),
                            )
                    for hd in range(4):
                        rope(Q_sb, hd, tb, qps[hd])
                    rope(K_sb, None, tb, kp)
                    vt_sb = ascr.tile([128, 512], dt.bfloat16, bufs=2)
                    nc.vector.tensor_copy(vt_sb[:], vtp[:])
                    for s4 in range(4):
                        nc.sync.dma_start_transpose(
                            V_sb[:, tb * 4 + s4, :],
                            vt_sb[:, s4 * 128 : (s4 + 1) * 128],
                        )

            # wo preload (hidden behind phase B)
            with tc.tile_pool(name="cw", bufs=1) as cw:
                wo_sb = cw.tile([128, 4, H], dt.bfloat16)
                nc.sync.dma_start(wo_sb[:], wo_d[:])

                # ---------------- Phase B: attention ----------------
                with (
                    tc.tile_pool(name="bpt", bufs=18) as bpt,
                    tc.tile_pool(name="bacc", bufs=2) as baccp,
                    tc.tile_pool(name="brp", bufs=3) as brp,
                    tc.tile_pool(name="brc", bufs=2) as brc,
                    tc.tile_pool(name="bbc", bufs=2) as bbc,
                    tc.tile_pool(name="pss", bufs=2, space="PSUM") as pss,
                    tc.tile_pool(name="pso", bufs=2, space="PSUM") as pso,
                    tc.tile_pool(name="psd", bufs=2, space="PSUM") as psd,
                ):
                    for b in range(2):
                        for hd in range(4):
                            r = b * 4 + hd
                            for g in range(4):
                                q0 = b * S + g * 512
                                nj = 4 * g + 4
                                pts = []
                                acc = baccp.tile([128, 512], dt.float32)
                                for j in range(nj):
                                    k0 = b * S + j * 128
                                    st = pss.tile([128, 512], dt.float32)
                                    nc.tensor.matmul(
                                        st[:],
                                        K_sb[:, k0 : k0 + 128],
                                        Q_sb[:, hd, q0 : q0 + 512],
                                        start=True,
                                        stop=True,
                                    )
                                    pt = bpt.tile([128, 512], dt.bfloat16)
                                    if j >= 4 * g:
                                        raw = brp.tile([128, 512], dt.bfloat16)
                                        nc.scalar.activation(
                                            raw[:], st[:], Act.Exp, scale=SCALE
                                        )
                                        # keep where q_global >= kv_global:
                                        # iota = (g*512 - j*128) - p + q_idx >= 0
                                        nc.gpsimd.affine_select(
                                            pt[:],
                                            raw[:],
                                            pattern=[[1, 512]],
                                            compare_op=Alu.is_ge,
                                            fill=0.0,
                                            base=g * 512 - j * 128,
                                            channel_multiplier=-1,
                                        )
                                    else:
                                        nc.scalar.activation(
                                            pt[:], st[:], Act.Exp, scale=SCALE
                                        )
                                    if j == 0:
                                        nc.vector.tensor_copy(acc[:], pt[:])
                                    else:
                                        nc.vector.tensor_add(acc[:], acc[:], pt[:])
                                    pts.append(pt)
                                dn = psd.tile([1, 512], dt.float32)
                                nc.tensor.matmul(
                                    dn[:], ones[:], acc[:], start=True, stop=True
                                )
                                rc = brc.tile([1, 512], dt.float32)
                                nc.vector.reciprocal(rc[:], dn[:])
                                nc.sync.dma_start(
                                    recip_d[r : r + 1, g * 512 : (g + 1) * 512], rc[:]
                                )
                                ot = pso.tile([128, 512], dt.float32)
                                for j in range(nj):
                                    nc.tensor.matmul(
                                        ot[:],
                                        V_sb[:, b * 16 + j, :],
                                        pts[j][:],
                                        start=(j == 0),
                                        stop=(j == nj - 1),
                                    )
                                nc.vector.tensor_copy(
                                    OT_sb[:, hd, q0 : q0 + 512], ot[:]
                                )

                    # normalize OT by 1/denom (broadcast rows from DRAM)
                    for b in range(2):
                        for hd in range(4):
                            r = b * 4 + hd
                            bc = bbc.tile([128, S], dt.float32)
                            src = bassmod.AP(
                                tensor=recip_d,
                                offset=r * S,
                                ap=[[0, 128], [1, S]],
                            )
                            nc.sync.dma_start(bc[:], src)
                            nc.vector.tensor_mul(
                                OT_sb[:, hd, b * S : (b + 1) * S],
                                OT_sb[:, hd, b * S : (b + 1) * S],
                                bc[:],
                            )

                # ---------------- Phase C: o_proj ----------------
                with (
                    tc.tile_pool(name="cy", bufs=2) as cy,
                    tc.tile_pool(name="psy", bufs=4, space="PSUM") as psy,
                ):
                    for i in range(T // 128):
                        ysb = cy.tile([128, H], dt.bfloat16)
                        for cb in range(8):
                            yp = psy.tile([128, 512], dt.float32)
                            for hd in range(4):
                                nc.tensor.matmul(
                                    yp[:],
                                    OT_sb[:, hd, i * 128 : (i + 1) * 128],
                                    wo_sb[:, hd, cb * 512 : (cb + 1) * 512],
                                    start=(hd == 0),
                                    stop=(hd == 3),
                                )
                            nc.scalar.activation(
                                ysb[:, cb * 512 : (cb + 1) * 512],
                                yp[:],
                                Act.Copy,
                                bias=0.0,
                            )
                        nc.sync.dma_start(y_d[i * 128 : (i + 1) * 128, :], ysb[:])

    nc.compile()
    return nc


def prep_inputs(inputs):
    hs = np.asarray(inputs["hidden_states"], np.float32)
    cos = np.asarray(inputs["cos"], np.float32)
    sin = np.asarray(inputs["sin"], np.float32)
    wq = np.asarray(inputs["wq"], np.float32)
    wk = np.asarray(inputs["wk"], np.float32)
    wv = np.asarray(inputs["wv"], np.float32)
    wo = np.asarray(inputs["wo"], np.float32)

    hsT = hs.reshape(T, H).T  # [H, T]
    hsT_p = hsT.reshape(KBLK, 128, T).transpose(1, 0, 2).astype(BF16)
    cosT = cos.transpose(2, 0, 1).reshape(128, T).astype(BF16)
    sinT = sin.transpose(2, 0, 1).reshape(128, T).astype(BF16)

    in_maps = []
    for c in range(NCORES):
        wq_c = wq[:, c * 512 : (c + 1) * 512]
        wk_c = wk[:, c * 128 : (c + 1) * 128]
        wv_c = wv[:, c * 128 : (c + 1) * 128]
        wo_c = wo[c * 512 : (c + 1) * 512, :]
        in_maps.append(
            {
                "hsT": hsT_p,
                "cosT": cosT,
                "sinT": sinT,
                "wq": wq_c.reshape(KBLK, 128, 512).transpose(1, 0, 2).astype(BF16),
                "wk": wk_c.reshape(KBLK, 128, 128).transpose(1, 0, 2).astype(BF16),
                "wv": wv_c.reshape(KBLK, 128, 128).transpose(1, 0, 2).astype(BF16),
                "wo": wo_c.reshape(4, 128, H).transpose(1, 0, 2).astype(BF16),
            }
        )
    return in_maps


def kernel(**inputs):
    global _NC
    from concourse.bass_utils import run_bass_kernel_spmd

    if _NC is None:
        _NC = build_nc()
    in_maps = prep_inputs(inputs)
    res = run_bass_kernel_spmd(_NC, in_maps, list(range(NCORES)))
    y = np.zeros((T, H), np.float32)
    for c in range(NCORES):
        y += res.results[c]["y"].astype(np.float32)
    return y.reshape(B, S, H).astype(np.float32)
